# revision 4
# baseline (speedup 1.0000x reference)
"""GridMask kernel for Trainium2, 8-core data parallel.

out[b,h,w,c] = x[b,h,w,c] * row_keep[b,h] * col_keep[b,w]

The grid mask is separable and zeroes ~50% of rows outright, so half the
input never influences the output. The kernel exploits that on the read
side: instead of streaming all of x through SBUF, a gpsimd indirect DMA
(SWDGE gather) loads ONLY the 4-row blocks that contain at least one
kept image row, using an index table computed host-side with exact
integer math. Fully-zeroed blocks keep a sentinel index that fails the
DGE bounds check and is silently skipped - no descriptor, no HBM
traffic. Probed HW semantics: the DGE emits ONE descriptor per
partition of the offset AP, start row = the partition's first index,
length = the dst partition's free size (6144 floats = rows 4p..4p+3);
an invalid first index leaves the whole partition untouched. The index
table stores all four per-row ids anyway so CoreSim (which honors one
index per 1536-float slot) produces bit-identical SBUF contents.
Skipped SBUF slots hold stale-but-finite data that the mask multiply
zeroes (buffers are memset once at start so no NaN bit patterns
survive).

Layout per core: 4 images, one SBUF tile per image [128, 6144] with
partition p holding image rows 4p..4p+3. The gather's flat destination
order (partition-major, 1536 floats per index) makes index i = 4p+r land
exactly at image row 4p+r's home slot, so the downstream mask-multiply
and store pipeline is unchanged from a dense kernel. The column mask is
broadcast to [128,1536] PSUM via a K=1 ones matmul; row mask enters the
STT as a per-partition scalar. Stores split across the sync and scalar
HW queues; gathers ride the gpsimd dynamic queue. All three queues
spread descriptors over the 16 shared DMA engines, whose aggregate
~380 GB/s descriptor-processing rate is the roofline: moving
12.6 MB out + ~6.3 MB in beats the dense kernel's 25.2 MB.
"""

import math

import numpy as np

import concourse.mybir as mybir
from concourse import bacc, bass, tile
from concourse.bass_utils import run_bass_kernel_spmd

B, H, W, C = 32, 512, 512, 3
D1 = 96
HH = math.ceil(math.sqrt(H * H + W * W))  # 725
OFF_H = (HH - H) // 2  # 106
OFF_W = (HH - W) // 2  # 106

NCORES = 8
BPC = B // NCORES  # images per core
FREE = W * C  # 1536 floats per image row

F32 = mybir.dt.float32
I32 = mybir.dt.int32

_CACHE: dict = {}


def _build_masks(d_raw, st_h_raw, st_w_raw):
    """Exact replica of the reference's integer mask math, in numpy."""
    d = D1 + d_raw.astype(np.int64)  # [B] stripe period
    l = (d + 1) // 2  # ceil(d * 0.5) for integer d
    st_h = st_h_raw.astype(np.int64) % d
    st_w = st_w_raw.astype(np.int64) % d
    yy = OFF_H + np.arange(H, dtype=np.int64)
    xx = OFF_W + np.arange(W, dtype=np.int64)
    row_zero = ((yy[None, :] - st_h[:, None]) % d[:, None]) < l[:, None]
    col_zero = ((xx[None, :] - st_w[:, None]) % d[:, None]) < l[:, None]
    row_keep = (~row_zero).astype(np.float32)  # [B,H]
    col_keep = (~col_zero).astype(np.float32)  # [B,W]
    return row_keep, col_keep


NTILES = BPC  # one image per tile
RPP = H // 128  # 4 consecutive image rows per partition
TILE_FREE = RPP * FREE  # 6144 floats = 24 KB per partition
NROWS = NTILES * H  # global gatherable rows per core (2048)
SENTINEL = 3000  # > NROWS-1: fails bounds check, row skipped


def _build_nc():
    nc = bacc.Bacc(None)
    # x as a flat row table [2048, 1536]: global row g = t*512 + r. The
    # indirect gather indexes axis 0 (coef = 1536 floats = one image row).
    x = nc.dram_tensor("x", [NROWS, FREE], F32, kind="ExternalInput")
    rowm = nc.dram_tensor("rowm", [128, NTILES * RPP], F32, kind="ExternalInput")
    # idx[p, t*RPP+r] = t*512 + 4p + r if that image row is kept, else
    # SENTINEL. Flat order (partition-major) matches the gather dst slots.
    idx = nc.dram_tensor("idx", [128, NTILES * RPP], I32, kind="ExternalInput")
    # col masks stay tiny in DRAM (one partition row); the TensorEngine
    # broadcasts them to [128, FREE] in PSUM via a K=1 ones matmul.
    colm = nc.dram_tensor("colm", [1, NTILES * FREE], F32, kind="ExternalInput")
    y = nc.dram_tensor("y", [NTILES, 128, TILE_FREE], F32, kind="ExternalOutput")

    mult = mybir.AluOpType.mult
    with tile.TileContext(nc) as tc:
        with (
            tc.tile_pool(name="const", bufs=1) as cpool,
            tc.tile_pool(name="io", bufs=NTILES) as iop,
            tc.tile_pool(name="psum", bufs=2, space="PSUM") as psp,
        ):
            rowm_sb = cpool.tile([128, NTILES * RPP], F32, tag="rowm")
            nc.sync.dma_start(rowm_sb[:], rowm[:])
            idx_sb = cpool.tile([128, NTILES * RPP], I32, tag="idx")
            nc.sync.dma_start(idx_sb[:], idx[:])
            colm_sb = cpool.tile([1, NTILES * FREE], F32, tag="colm")
            nc.sync.dma_start(colm_sb[:], colm[:])
            ones_sb = cpool.tile([1, 128], F32, tag="ones")
            nc.vector.memset(ones_sb[:], 1.0)

            # One-time zero of every io buffer: skipped gather slots must
            # hold finite data so stale-NaN * 0 can't poison the output.
            tiles = []
            for t in range(NTILES):
                xt = iop.tile([128, TILE_FREE], F32, tag="xt")
                nc.vector.memset(xt[:], 0.0)
                tiles.append(xt)

            for t in range(NTILES):
                xt = tiles[t]
                # Gather only the kept rows of image t into home slots.
                nc.gpsimd.indirect_dma_start(
                    out=xt[:],
                    out_offset=None,
                    in_=x[:],
                    in_offset=bass.IndirectOffsetOnAxis(
                        ap=idx_sb[:, t * RPP : (t + 1) * RPP],
                        axis=0,
                    ),
                    bounds_check=NROWS - 1,
                    oob_is_err=False,
                )
                cmask = psp.tile([128, FREE], F32, tag="cmask")
                for ch in range(FREE // 512):
                    sl = slice(t * FREE + ch * 512, t * FREE + (ch + 1) * 512)
                    nc.tensor.matmul(
                        cmask[:, ch * 512 : (ch + 1) * 512],
                        ones_sb[:],
                        colm_sb[:, sl],
                        start=True,
                        stop=True,
                    )
                for r in range(RPP):
                    rs = slice(r * FREE, (r + 1) * FREE)
                    nc.vector.scalar_tensor_tensor(
                        xt[:, rs],
                        xt[:, rs],
                        rowm_sb[:, t * RPP + r : t * RPP + r + 1],
                        cmask[:],
                        op0=mult,
                        op1=mult,
                    )
                # Split the 12.6 MB of stores across both HW DGE queues.
                if t % 2 == 0:
                    nc.sync.dma_start(y[t], xt[:])
                else:
                    nc.scalar.dma_start(y[t], xt[:])
    nc.compile()
    return nc


def _prep_inputs(x, d_raw, st_h_raw, st_w_raw):
    x = np.ascontiguousarray(np.asarray(x, dtype=np.float32))
    row_keep, col_keep = _build_masks(
        np.asarray(d_raw), np.asarray(st_h_raw), np.asarray(st_w_raw)
    )
    col_exp = np.repeat(col_keep, C, axis=1)  # [B, W*C]
    # Global row ids per core: g = t*H + h, laid out [128, NTILES*RPP] with
    # idx[p, t*RPP+r] describing image t's row 4p+r.
    g = np.arange(H, dtype=np.int32)
    in_maps = []
    for c in range(NCORES):
        sl = slice(c * BPC, (c + 1) * BPC)
        xc = x[sl].reshape(NROWS, FREE)
        rk = row_keep[sl].astype(bool)  # [NTILES, H]
        # Block granularity: gather rows 4p..4p+3 iff any of them is kept
        # (the dead rows in a fetched block are zeroed by the rowm STT).
        blk = rk.reshape(NTILES, 128, RPP).any(axis=2, keepdims=True)  # [NTILES,128,1]
        blk = np.broadcast_to(blk, (NTILES, 128, RPP)).reshape(NTILES, H)
        idx = np.where(blk, g[None, :] + (np.arange(NTILES, dtype=np.int32) * H)[:, None], SENTINEL)
        # [NTILES, H] -> [NTILES, 128, RPP] -> [128, NTILES, RPP]
        idx = np.ascontiguousarray(
            idx.reshape(NTILES, 128, RPP).transpose(1, 0, 2).reshape(128, NTILES * RPP)
        ).astype(np.int32)
        rm = np.ascontiguousarray(
            row_keep[sl]
            .reshape(NTILES, 128, RPP)
            .transpose(1, 0, 2)
            .reshape(128, NTILES * RPP)
        )
        cm = np.ascontiguousarray(col_exp[sl].reshape(1, NTILES * FREE))
        in_maps.append({"x": xc, "rowm": rm, "idx": idx, "colm": cm})
    return in_maps


def kernel(x, d_raw, st_h_raw, st_w_raw):
    if "nc" not in _CACHE:
        _CACHE["nc"] = _build_nc()
    nc = _CACHE["nc"]
    in_maps = _prep_inputs(x, d_raw, st_h_raw, st_w_raw)
    res = run_bass_kernel_spmd(nc, in_maps, list(range(NCORES)))
    out = np.concatenate(
        [np.asarray(r["y"]).reshape(BPC, H, W, C) for r in res.results], axis=0
    )
    return out


# revision 5
# speedup vs baseline: 1.0694x; 1.0694x over previous
"""GridMask kernel for Trainium2, 8-core data parallel.

out[b,h,w,c] = x[b,h,w,c] * row_keep[b,h] * col_keep[b,w]

The grid mask is separable and zeroes ~50% of rows outright, so half the
input never influences the output. The kernel exploits that on the read
side: a gpsimd indirect DMA (SWDGE gather) loads ONLY the 4-row blocks
that contain at least one kept image row, using an index table computed
host-side with exact integer math. Fully-zeroed blocks keep a sentinel
index that fails the DGE bounds check and is skipped - no descriptor,
no HBM traffic. Probed HW semantics: the DGE emits ONE descriptor per
partition of the offset AP, start row = the partition's first index,
length = the dst partition's free size (6144 floats = rows 4p..4p+3);
an invalid index leaves the whole partition untouched (hole). Holes and
the dead rows inside fetched blocks are zeroed by the row-mask STT;
io buffers are memset once so pre-kernel SBUF NaNs can't survive a
0-multiply (3-buffer rotation: the 4th image reuses buffer 0 whose
stale contents are then finite image data).

Per core: 4 images, SBUF tile per image [128, 6144], partition p =
image rows 4p..4p+3 (24 KB descriptors, the DMA engines' best rate).
The column mask is broadcast to [128,1536] PSUM via K=1 bf16 matmuls;
row mask enters the STT as a per-partition scalar. Gathers ride the
gpsimd dynamic queue; each image's store is split across the sync and
scalar HW queues (halves) so no single queue carries the write tail.
All queues spread descriptors over the 16 shared DMA engines
(~25 GB/s/engine): moving ~6.8 MB in + 12.6 MB out beats the dense
kernel's 25.2 MB.
"""

import math

import numpy as np

import concourse.mybir as mybir
from concourse import bacc, bass, tile
from concourse.bass_utils import run_bass_kernel_spmd

B, H, W, C = 32, 512, 512, 3
D1 = 96
HH = math.ceil(math.sqrt(H * H + W * W))  # 725
OFF_H = (HH - H) // 2  # 106
OFF_W = (HH - W) // 2  # 106

NCORES = 8
BPC = B // NCORES  # images per core
FREE = W * C  # 1536 floats per image row

F32 = mybir.dt.float32
BF16 = mybir.dt.bfloat16
I32 = mybir.dt.int32

_CACHE: dict = {}


def _build_masks(d_raw, st_h_raw, st_w_raw):
    """Exact replica of the reference's integer mask math, in numpy."""
    d = D1 + d_raw.astype(np.int64)  # [B] stripe period
    l = (d + 1) // 2  # ceil(d * 0.5) for integer d
    st_h = st_h_raw.astype(np.int64) % d
    st_w = st_w_raw.astype(np.int64) % d
    yy = OFF_H + np.arange(H, dtype=np.int64)
    xx = OFF_W + np.arange(W, dtype=np.int64)
    row_zero = ((yy[None, :] - st_h[:, None]) % d[:, None]) < l[:, None]
    col_zero = ((xx[None, :] - st_w[:, None]) % d[:, None]) < l[:, None]
    row_keep = (~row_zero).astype(np.float32)  # [B,H]
    col_keep = (~col_zero).astype(np.float32)  # [B,W]
    return row_keep, col_keep


NTILES = BPC  # one image per tile
RPP = H // 128  # 4 consecutive image rows per partition
TILE_FREE = RPP * FREE  # 6144 floats = 24 KB per partition
NROWS = NTILES * H  # global gatherable rows per core (2048)
SENTINEL = 3000  # > NROWS-1: fails bounds check, block skipped
NBUFS = 3  # io tile rotation depth (image 3 reuses buffer 0)


def _build_nc():
    nc = bacc.Bacc(None)
    # x as a flat row table [2048, 1536]: global row g = t*512 + r. The
    # indirect gather indexes axis 0 (coef = 1536 floats = one image row);
    # each valid index fetches a whole 4-row block (dst partition size).
    x = nc.dram_tensor("x", [NROWS, FREE], F32, kind="ExternalInput")
    rowm = nc.dram_tensor("rowm", [128, NTILES * RPP], F32, kind="ExternalInput")
    # idx[p, t] = t*512 + 4p if any of image t's rows 4p..4p+3 kept else SENTINEL
    idx = nc.dram_tensor("idx", [128, NTILES], I32, kind="ExternalInput")
    # col masks, bf16 (0/1 exact): broadcast via K=1 matmul to [128,FREE] PSUM
    colm = nc.dram_tensor("colm", [1, NTILES * FREE], BF16, kind="ExternalInput")
    y = nc.dram_tensor("y", [NTILES, 128, TILE_FREE], F32, kind="ExternalOutput")

    mult = mybir.AluOpType.mult
    with tile.TileContext(nc) as tc:
        with (
            tc.tile_pool(name="const", bufs=1) as cpool,
            tc.tile_pool(name="io", bufs=NBUFS) as iop,
            tc.tile_pool(name="psum", bufs=2, space="PSUM") as psp,
        ):
            idx_sb = cpool.tile([128, NTILES], I32, tag="idx")
            nc.sync.dma_start(idx_sb[:], idx[:])
            rowm_sb = cpool.tile([128, NTILES * RPP], F32, tag="rowm")
            nc.sync.dma_start(rowm_sb[:], rowm[:])
            colm_sb = cpool.tile([1, NTILES * FREE], BF16, tag="colm")
            nc.sync.dma_start(colm_sb[:], colm[:])
            ones_sb = cpool.tile([1, 128], BF16, tag="ones")
            nc.vector.memset(ones_sb[:], 1.0)

            # One-time zero of the io buffers (first use of each): skipped
            # gather slots must hold finite data so stale-NaN * 0 can't
            # poison the output. Split across DVE and gpsimd so neither
            # engine serializes both before the first gather.
            tiles = []
            for i in range(NBUFS):
                xt = iop.tile([128, TILE_FREE], F32, tag="xt")
                (nc.vector if i != 1 else nc.gpsimd).memset(xt[:], 0.0)
                tiles.append(xt)

            for t in range(NTILES):
                if t < NBUFS:
                    xt = tiles[t]
                else:
                    xt = iop.tile([128, TILE_FREE], F32, tag="xt")
                # Gather only the live 4-row blocks of image t into home slots.
                nc.gpsimd.indirect_dma_start(
                    out=xt[:],
                    out_offset=None,
                    in_=x[:],
                    in_offset=bass.IndirectOffsetOnAxis(
                        ap=idx_sb[:, t : t + 1],
                        axis=0,
                    ),
                    bounds_check=NROWS - 1,
                    oob_is_err=False,
                )
                cmask = psp.tile([128, FREE], F32, tag="cmask")
                for ch in range(FREE // 512):
                    sl = slice(t * FREE + ch * 512, t * FREE + (ch + 1) * 512)
                    nc.tensor.matmul(
                        cmask[:, ch * 512 : (ch + 1) * 512],
                        ones_sb[:],
                        colm_sb[:, sl],
                        start=True,
                        stop=True,
                    )
                for r in range(RPP):
                    rs = slice(r * FREE, (r + 1) * FREE)
                    nc.vector.scalar_tensor_tensor(
                        xt[:, rs],
                        xt[:, rs],
                        rowm_sb[:, t * RPP + r : t * RPP + r + 1],
                        cmask[:],
                        op0=mult,
                        op1=mult,
                    )
                # Split each image's 3.15 MB store across both HW DGE queues.
                half = TILE_FREE // 2
                nc.sync.dma_start(y[t][:, :half], xt[:, :half])
                nc.scalar.dma_start(y[t][:, half:], xt[:, half:])
    nc.compile()
    return nc


def _prep_inputs(x, d_raw, st_h_raw, st_w_raw):
    x = np.ascontiguousarray(np.asarray(x, dtype=np.float32))
    row_keep, col_keep = _build_masks(
        np.asarray(d_raw), np.asarray(st_h_raw), np.asarray(st_w_raw)
    )
    col_exp = np.repeat(col_keep, C, axis=1)  # [B, W*C]
    in_maps = []
    for c in range(NCORES):
        sl = slice(c * BPC, (c + 1) * BPC)
        xc = x[sl].reshape(NROWS, FREE)
        rk = row_keep[sl].astype(bool)  # [NTILES, H]
        # Block granularity: gather rows 4p..4p+3 iff any of them is kept
        # (the dead rows in a fetched block are zeroed by the rowm STT).
        blk = rk.reshape(NTILES, 128, RPP).any(axis=2)  # [NTILES, 128]
        base = (np.arange(NTILES, dtype=np.int32) * H)[:, None] + (
            np.arange(128, dtype=np.int32) * RPP
        )[None, :]
        idx = np.where(blk, base, SENTINEL).astype(np.int32)
        idx = np.ascontiguousarray(idx.T)  # [128, NTILES]
        rm = np.ascontiguousarray(
            row_keep[sl]
            .reshape(NTILES, 128, RPP)
            .transpose(1, 0, 2)
            .reshape(128, NTILES * RPP)
        )
        cm = np.ascontiguousarray(col_exp[sl].reshape(1, NTILES * FREE)).astype(
            mybir.dt.np(BF16)
        )
        in_maps.append({"x": xc, "rowm": rm, "idx": idx, "colm": cm})
    return in_maps


def kernel(x, d_raw, st_h_raw, st_w_raw):
    if "nc" not in _CACHE:
        _CACHE["nc"] = _build_nc()
    nc = _CACHE["nc"]
    in_maps = _prep_inputs(x, d_raw, st_h_raw, st_w_raw)
    res = run_bass_kernel_spmd(nc, in_maps, list(range(NCORES)))
    out = np.concatenate(
        [np.asarray(r["y"]).reshape(BPC, H, W, C) for r in res.results], axis=0
    )
    return out


# revision 8
# speedup vs baseline: 1.1171x; 1.0445x over previous
"""GridMask kernel for Trainium2, 8-core data parallel.

out[b,h,w,c] = x[b,h,w,c] * row_keep[b,h] * col_keep[b,w]

The grid mask is separable and zeroes ~50% of rows outright, so half the
input never influences the output and half the output is all-zero rows.
The kernel exploits both sides:

- Read side: a gpsimd indirect DMA (SWDGE gather) loads ONLY the 4-row
  blocks containing at least one kept image row, via an index table
  computed host-side with exact integer math. Fully-dead blocks carry a
  sentinel index that fails the DGE bounds check and is skipped - no
  descriptor, no HBM traffic. Probed HW semantics: one descriptor per
  partition of the offset AP, start row = the partition's first index,
  length = dst partition free size (6144 floats = rows 4p..4p+3);
  invalid index = untouched partition.
- Write side: the Bass runtime hands every ExternalOutput to the kernel
  pre-zeroed (both the native path, which np.zeros()es output buffers,
  and the PJRT path, which donates zero buffers - kernels that don't
  write every element rely on this contract). So the all-zero blocks of
  y are never written: an indirect SCATTER with the same index table
  writes back only the live blocks, and sentinel partitions leave the
  pre-zeroed rows alone.

Stale SBUF data in dead blocks never reaches y (never scattered), so no
buffer initialization is needed at all. Dead rows inside live blocks are
zeroed by the row-mask STT, exactly as the reference's 0*x.

Per core: 4 images, SBUF tile per image [128, 6144], partition p =
image rows 4p..4p+3 (24 KB descriptors, the DMA engines' best rate).
The column mask (bf16 0/1, exact) is broadcast to [128,1536] f32 PSUM
via K=1 matmuls; the row mask enters the STT as a per-partition scalar.
Total HBM traffic: ~6.8 MB in + ~6.8 MB out per core, vs the dense
kernel's 25.2 MB.
"""

import math

import numpy as np

import concourse.mybir as mybir
from concourse import bacc, bass, tile
from concourse.bass_utils import run_bass_kernel_spmd

B, H, W, C = 32, 512, 512, 3
D1 = 96
HH = math.ceil(math.sqrt(H * H + W * W))  # 725
OFF_H = (HH - H) // 2  # 106
OFF_W = (HH - W) // 2  # 106

NCORES = 8
BPC = B // NCORES  # images per core
FREE = W * C  # 1536 floats per image row

F32 = mybir.dt.float32
BF16 = mybir.dt.bfloat16
I32 = mybir.dt.int32

_CACHE: dict = {}


def _build_masks(d_raw, st_h_raw, st_w_raw):
    """Exact replica of the reference's integer mask math, in numpy."""
    d = D1 + d_raw.astype(np.int64)  # [B] stripe period
    l = (d + 1) // 2  # ceil(d * 0.5) for integer d
    st_h = st_h_raw.astype(np.int64) % d
    st_w = st_w_raw.astype(np.int64) % d
    yy = OFF_H + np.arange(H, dtype=np.int64)
    xx = OFF_W + np.arange(W, dtype=np.int64)
    row_zero = ((yy[None, :] - st_h[:, None]) % d[:, None]) < l[:, None]
    col_zero = ((xx[None, :] - st_w[:, None]) % d[:, None]) < l[:, None]
    row_keep = (~row_zero).astype(np.float32)  # [B,H]
    col_keep = (~col_zero).astype(np.float32)  # [B,W]
    return row_keep, col_keep


NTILES = BPC  # one image per tile
RPP = H // 128  # 4 consecutive image rows per partition
TILE_FREE = RPP * FREE  # 6144 floats = 24 KB per partition
NROWS = NTILES * H  # global row count per core (2048)
SENTINEL = 3000  # > NROWS-1: fails bounds check, block skipped
NBUFS = 4  # one io tile per image: no write-after-read stalls


def _build_nc():
    nc = bacc.Bacc(None)
    # x and y as flat row tables [2048, 1536]: global row g = t*512 + r.
    # Indirect gather/scatter index axis 0 (coef = 1536 floats = one row);
    # each valid index moves a whole 4-row block (partition free size).
    x = nc.dram_tensor("x", [NROWS, FREE], F32, kind="ExternalInput")
    rowm = nc.dram_tensor("rowm", [128, NTILES * RPP], F32, kind="ExternalInput")
    # idx[p, t] = t*512 + 4p if any of image t's rows 4p..4p+3 kept else SENTINEL
    idx = nc.dram_tensor("idx", [128, NTILES], I32, kind="ExternalInput")
    # col masks, bf16 (0/1 exact): broadcast via K=1 matmul to [128,FREE] PSUM
    colm = nc.dram_tensor("colm", [1, NTILES * FREE], BF16, kind="ExternalInput")
    y = nc.dram_tensor("y", [NROWS, FREE], F32, kind="ExternalOutput")

    mult = mybir.AluOpType.mult
    with tile.TileContext(nc) as tc:
        with (
            tc.tile_pool(name="const", bufs=1) as cpool,
            tc.tile_pool(name="io", bufs=NBUFS) as iop,
            tc.tile_pool(name="psum", bufs=2, space="PSUM") as psp,
        ):
            idx_sb = cpool.tile([128, NTILES], I32, tag="idx")
            nc.sync.dma_start(idx_sb[:], idx[:])
            rowm_sb = cpool.tile([128, NTILES * RPP], F32, tag="rowm")
            nc.sync.dma_start(rowm_sb[:], rowm[:])
            colm_sb = cpool.tile([1, NTILES * FREE], BF16, tag="colm")
            nc.sync.dma_start(colm_sb[:], colm[:])
            ones_sb = cpool.tile([1, 128], BF16, tag="ones")
            nc.vector.memset(ones_sb[:], 1.0)

            for t in range(NTILES):
                xt = iop.tile([128, TILE_FREE], F32, tag="xt", name=f"xt{t}")
                # Gather only the live 4-row blocks of image t into home slots.
                nc.gpsimd.indirect_dma_start(
                    out=xt[:],
                    out_offset=None,
                    in_=x[:],
                    in_offset=bass.IndirectOffsetOnAxis(
                        ap=idx_sb[:, t : t + 1],
                        axis=0,
                    ),
                    bounds_check=NROWS - 1,
                    oob_is_err=False,
                )
                cmask = psp.tile([128, FREE], F32, tag="cmask", name=f"cm{t}")
                for ch in range(FREE // 512):
                    sl = slice(t * FREE + ch * 512, t * FREE + (ch + 1) * 512)
                    nc.tensor.matmul(
                        cmask[:, ch * 512 : (ch + 1) * 512],
                        ones_sb[:],
                        colm_sb[:, sl],
                        start=True,
                        stop=True,
                    )
                for r in range(RPP):
                    rs = slice(r * FREE, (r + 1) * FREE)
                    nc.vector.scalar_tensor_tensor(
                        xt[:, rs],
                        xt[:, rs],
                        rowm_sb[:, t * RPP + r : t * RPP + r + 1],
                        cmask[:],
                        op0=mult,
                        op1=mult,
                    )
                # Scatter the live blocks back; sentinel partitions skip,
                # leaving the runtime's pre-zeroed y rows in place.
                nc.gpsimd.indirect_dma_start(
                    out=y[:],
                    out_offset=bass.IndirectOffsetOnAxis(
                        ap=idx_sb[:, t : t + 1],
                        axis=0,
                    ),
                    in_=xt[:],
                    in_offset=None,
                    bounds_check=NROWS - 1,
                    oob_is_err=False,
                )
    nc.compile()
    return nc


def _prep_inputs(x, d_raw, st_h_raw, st_w_raw):
    x = np.ascontiguousarray(np.asarray(x, dtype=np.float32))
    row_keep, col_keep = _build_masks(
        np.asarray(d_raw), np.asarray(st_h_raw), np.asarray(st_w_raw)
    )
    col_exp = np.repeat(col_keep, C, axis=1)  # [B, W*C]
    in_maps = []
    for c in range(NCORES):
        sl = slice(c * BPC, (c + 1) * BPC)
        xc = x[sl].reshape(NROWS, FREE)
        rk = row_keep[sl].astype(bool)  # [NTILES, H]
        # Block granularity: move rows 4p..4p+3 iff any of them is kept
        # (the dead rows in a live block are zeroed by the rowm STT).
        blk = rk.reshape(NTILES, 128, RPP).any(axis=2)  # [NTILES, 128]
        base = (np.arange(NTILES, dtype=np.int32) * H)[:, None] + (
            np.arange(128, dtype=np.int32) * RPP
        )[None, :]
        idx = np.where(blk, base, SENTINEL).astype(np.int32)
        idx = np.ascontiguousarray(idx.T)  # [128, NTILES]
        rm = np.ascontiguousarray(
            row_keep[sl]
            .reshape(NTILES, 128, RPP)
            .transpose(1, 0, 2)
            .reshape(128, NTILES * RPP)
        )
        cm = np.ascontiguousarray(col_exp[sl].reshape(1, NTILES * FREE)).astype(
            mybir.dt.np(BF16)
        )
        in_maps.append({"x": xc, "rowm": rm, "idx": idx, "colm": cm})
    return in_maps


def kernel(x, d_raw, st_h_raw, st_w_raw):
    if "nc" not in _CACHE:
        _CACHE["nc"] = _build_nc()
    nc = _CACHE["nc"]
    in_maps = _prep_inputs(x, d_raw, st_h_raw, st_w_raw)
    res = run_bass_kernel_spmd(nc, in_maps, list(range(NCORES)))
    out = np.concatenate(
        [np.asarray(r["y"]).reshape(BPC, H, W, C) for r in res.results], axis=0
    )
    return out
